# revision 13
# baseline (speedup 1.0000x reference)
"""Trainium2 Bass kernel for DigitConvolutionalModel.

Model: x[B,784] -> 3x3 valid conv (1 channel) -> flatten(676) -> FC(128)+relu
       -> FC(128)+relu (same W2 twice) -> FC(10).

Strategy:
  * Conv is linear, so it is folded into W1 on the host: W1f = C@W1 with C the
    [784,676] conv operator; the network becomes 4 dense layers.
  * Pure data parallel: batch 65536 split as 8192 per NeuronCore.
  * Activations stay transposed on chip ([hid partitions, batch free dim]);
    every layer is out = lhsT.T @ rhs with lhsT = weights.
  * The kernel is DMA-bound: 8 cores streaming simultaneously sustain only
    ~260 GB/s/core of HBM->SBUF, so input bytes are the roofline. Mixed
    precision cuts them: the PE accepts bf16 stationary x fp8 moving operands
    natively (verified exact), so the 448 input features with the smallest
    W1f row norms travel as fp8e4m3 and the sensitive 336 as bf16
    (0.714x bytes, rel err ~1.5e-2 vs 2e-2 budget). Weights stay bf16.
  * Per chunk, the 4 fp8 tiles + 3 bf16 tiles are packed into one uint8 DRAM
    row per partition (~5KB rows - measured near the per-row DMA sweet spot),
    one dma_start per chunk on the sync HWDGE ring; SBUF slices are
    bitcast back to fp8/bf16 for the matmuls.
  * Small first/last chunks shorten the serial dependency tail; outputs are
    flushed in 1024-col pieces on the sync ring (idle after input triggers).
  * 4-stage software pipeline as before: PE runs L1(s),L2(s-1),L3(s-2),
    L4(s-3); ACT does relu+bias for L1/L3, DVE for L2/L4.
"""

import os
import sys

sys.path.insert(0, "/opt/trn_rl_repo")

import ml_dtypes
import numpy as np

import concourse.bacc as bacc
import concourse.mybir as mybir
import concourse.tile as tile
from concourse.bass_utils import run_bass_kernel_spmd

B = 65536
IN_SIDE = 28
KSZ = 3
OUT_SIDE = IN_SIDE - KSZ + 1  # 26
FLAT = OUT_SIDE * OUT_SIDE  # 676
IN_FLAT = IN_SIDE * IN_SIDE  # 784
HID = 128
OUT = 10

N_CORES = 8
B_SHARD = B // N_CORES  # 8192
KP = 112  # feature-tile partition size (784 = 7*112)
KT = IN_FLAT // KP  # 7
NT8 = int(os.environ.get("KERNEL_NT8", "4"))  # tiles sent as fp8e4m3
NT16 = KT - NT8  # tiles sent as bf16
ROW_B_PER_COL = NT8 + 2 * NT16  # uint8 row bytes per batch column

# chunk schedule (batch columns per chunk): 1024-col bulk chunks give
# 10240B DMA rows (measured at the ~260GB/s per-row sweet spot) and hold
# exactly two 512-col matmul groups; small tail chunks shorten the final
# serial dependency chain
_chunks = os.environ.get("KERNEL_CHUNKS", "512,7x1024,256,256")
CHUNKS = []
for part in _chunks.split(","):
    if "x" in part:
        n, v = part.split("x")
        CHUNKS += [int(v)] * int(n)
    else:
        CHUNKS.append(int(part))
assert sum(CHUNKS) == B_SHARD
CHUNK_OFF = np.concatenate([[0], np.cumsum(CHUNKS)]).tolist()
# matmul groups: 512-col slices, never crossing a chunk boundary
GROUPS = []  # (chunk, local col offset, width)
for c, cw in enumerate(CHUNKS):
    off = 0
    while off < cw:
        w = min(512, cw - off)
        GROUPS.append((c, off, w))
        off += w

X_BUFS = int(os.environ.get("KERNEL_X_BUFS", "0"))  # 0 => all chunks resident

# packed weight row layout (bytes per partition row of the wt tensor):
# w1 [112p, 7 tiles x 256B] | w2 [128p, 256B] | w3 [128p, 20B] |
# b1 f32 | b2 f32 | b3 f32 (10p)
W1_B = KT * HID * 2  # 3584
W2_OFF = W1_B
W3_OFF = W2_OFF + HID * 2  # 3840
B1_OFF = W3_OFF + OUT * 2 + 16  # 3876 (16B pad keeps 4B alignment roomy)
B2_OFF = B1_OFF + 4
B3_OFF = B2_OFF + 4
WT_B = B3_OFF + 4  # 3888
H_BUFS = int(os.environ.get("KERNEL_H_BUFS", "4"))
PS_BUFS = int(os.environ.get("KERNEL_PS_BUFS", "8"))
OUT_FLUSH = int(os.environ.get("KERNEL_OUT_FLUSH", "1024"))
_skew = os.environ.get("KERNEL_SKEW", "2,4,6")
D2, D3, D4 = [int(v) for v in _skew.split(",")]

BF16 = mybir.dt.bfloat16
F8 = mybir.dt.float8e4
U8 = mybir.dt.uint8
F32 = mybir.dt.float32

LAST_EXEC_NS = None
LAST_RESULTS = None

_compiled = {}


def _build_program():
    n_chunks = len(CHUNKS)
    xt_bytes = ROW_B_PER_COL * B_SHARD

    nc = bacc.Bacc(
        "TRN2", target_bir_lowering=False, debug=False, num_devices=N_CORES
    )
    # packed input: chunk c spans byte cols
    # [ROW_B_PER_COL*off_c, ROW_B_PER_COL*off_{c+1}) ; within a chunk of C
    # cols the row is [4 fp8 tiles of C bytes | 3 bf16 tiles of 2C bytes]
    xt = nc.dram_tensor("xt", [KP, xt_bytes], U8, kind="ExternalInput")
    wt = nc.dram_tensor("wt", [HID, WT_B], U8, kind="ExternalInput")
    yt = nc.dram_tensor("yt", [OUT, B_SHARD], F32, kind="ExternalOutput")

    Relu = mybir.ActivationFunctionType.Relu
    add = mybir.AluOpType.add
    amax = mybir.AluOpType.max

    x_bufs = X_BUFS if X_BUFS > 0 else n_chunks
    with tile.TileContext(nc) as tc:
        with (
            tc.tile_pool(name="wpool", bufs=1) as wpool,
            tc.tile_pool(name="xpool", bufs=x_bufs) as xpool,
            tc.tile_pool(name="hpool", bufs=H_BUFS) as hpool,
            tc.tile_pool(name="opool", bufs=1) as opool,
            tc.tile_pool(name="psum", bufs=PS_BUFS, space="PSUM") as pp,
        ):
            # all weights+biases packed in one [128, 3888B] tensor on the
            # scalar (ACT) HWDGE ring so it lands in parallel with chunk 0
            # streaming on the sync ring; the SWDGE path used before took
            # ~12us of tiny descriptors and gated the PE start
            wt_sb = wpool.tile([HID, WT_B], U8)
            nc.scalar.dma_start(out=wt_sb[:], in_=wt.ap())

            def w1_tile(k):
                return wt_sb[0:KP, k * HID * 2 : (k + 1) * HID * 2].bitcast(
                    BF16
                )

            w2_sb = wt_sb[:, W2_OFF : W2_OFF + HID * 2].bitcast(BF16)
            w3_sb = wt_sb[:, W3_OFF : W3_OFF + OUT * 2].bitcast(BF16)
            b1_sb = wt_sb[:, B1_OFF : B1_OFF + 4].bitcast(F32)
            b2_sb = wt_sb[:, B2_OFF : B2_OFF + 4].bitcast(F32)
            b3_sb = wt_sb[0:OUT, B3_OFF : B3_OFF + 4].bitcast(F32)

            yt_sb = opool.tile([OUT, B_SHARD], F32)

            xt_tiles = []
            for c in range(n_chunks):
                cw = CHUNKS[c]
                xt_sb = xpool.tile([KP, ROW_B_PER_COL * cw], U8, tag="xt")
                xt_tiles.append(xt_sb)
            for c in range(n_chunks):
                b0 = ROW_B_PER_COL * CHUNK_OFF[c]
                b1_ = ROW_B_PER_COL * CHUNK_OFF[c + 1]
                nc.sync.dma_start(out=xt_tiles[c][:], in_=xt.ap()[:, b0:b1_])

            def rhs_l1(g, k):
                """L1 moving operand: tile k of group g, bitcast view."""
                c, off, w = GROUPS[g]
                cw = CHUNKS[c]
                t = xt_tiles[c]
                if k < NT8:
                    lo = k * cw + off
                    return t[:, lo : lo + w].bitcast(F8)
                lo = NT8 * cw + 2 * ((k - NT8) * cw + off)
                return t[:, lo : lo + 2 * w].bitcast(BF16)

            def gw(g):
                return GROUPS[g][2]

            def goff(g):
                c, off, _ = GROUPS[g]
                return CHUNK_OFF[c] + off

            n_sub = len(GROUPS)
            h1t = {}
            h2t = {}
            h3t = {}
            ps = {}
            out_flushed = 0

            # 4-stage software pipeline: at skew step s the PE runs
            # L1(s), L2(s-D2), L3(s-D3), L4(s-D4) back-to-back; ACT/DVE
            # trail one stage behind each matmul.
            for s in range(n_sub + D4):
                if s < n_sub:
                    ps1 = pp.tile([HID, gw(s)], F32, tag="ps")
                    for k in range(KT):
                        nc.tensor.matmul(
                            ps1[:],
                            w1_tile(k),
                            rhs_l1(s, k),
                            start=(k == 0),
                            stop=(k == KT - 1),
                        )
                    ps[("1", s)] = ps1
                if s >= D2 and (s - D2) < n_sub:
                    j = s - D2
                    ps2 = pp.tile([HID, gw(j)], F32, tag="ps")
                    nc.tensor.matmul(
                        ps2[:], w2_sb, h1t[j][:], start=True, stop=True
                    )
                    ps[("2", j)] = ps2
                if s >= D3 and (s - D3) < n_sub:
                    j = s - D3
                    ps3 = pp.tile([HID, gw(j)], F32, tag="ps")
                    nc.tensor.matmul(
                        ps3[:], w2_sb, h2t[j][:], start=True, stop=True
                    )
                    ps[("3", j)] = ps3
                if s >= D4 and (s - D4) < n_sub:
                    j = s - D4
                    ps4 = pp.tile([OUT, gw(j)], F32, tag="ps")
                    nc.tensor.matmul(
                        ps4[:], w3_sb, h3t[j][:], start=True, stop=True
                    )
                    ps[("4", j)] = ps4

                # trailing activation/bias stages (other engines)
                if s < n_sub:
                    h1 = hpool.tile([HID, gw(s)], BF16, tag="h1")
                    nc.scalar.activation(
                        h1[:], ps[("1", s)][:], Relu, bias=b1_sb
                    )
                    h1t[s] = h1
                if s >= D2 and (s - D2) < n_sub:
                    j = s - D2
                    h2 = hpool.tile([HID, gw(j)], BF16, tag="h2")
                    nc.vector.tensor_scalar(
                        out=h2[:],
                        in0=ps[("2", j)][:],
                        scalar1=b2_sb,
                        scalar2=0.0,
                        op0=add,
                        op1=amax,
                    )
                    h2t[j] = h2
                if s >= D3 and (s - D3) < n_sub:
                    j = s - D3
                    h3 = hpool.tile([HID, gw(j)], BF16, tag="h3")
                    nc.scalar.activation(
                        h3[:], ps[("3", j)][:], Relu, bias=b2_sb
                    )
                    h3t[j] = h3
                if s >= D4 and (s - D4) < n_sub:
                    j = s - D4
                    j0 = goff(j)
                    nc.vector.tensor_scalar(
                        out=yt_sb[:, j0 : j0 + gw(j)],
                        in0=ps[("4", j)][:],
                        scalar1=b3_sb,
                        scalar2=None,
                        op0=add,
                    )
                    # flush finished cols on the sync ring (idle once the
                    # input triggers are all issued)
                    done = j0 + gw(j)
                    if done - out_flushed >= OUT_FLUSH or j == n_sub - 1:
                        nc.sync.dma_start(
                            out=yt.ap()[:, out_flushed:done],
                            in_=yt_sb[:, out_flushed:done],
                        )
                        out_flushed = done

    nc.compile()
    return nc


def _fold_conv_into_w1(conv_w, W1):
    """W1f[784,128] such that x @ W1f == conv(x).flatten @ W1."""
    W1_img = np.asarray(W1, np.float64).reshape(OUT_SIDE, OUT_SIDE, HID)
    cw = np.asarray(conv_w, np.float64).reshape(KSZ, KSZ)
    W1f = np.zeros((IN_SIDE, IN_SIDE, HID), np.float64)
    for di in range(KSZ):
        for dj in range(KSZ):
            W1f[di : di + OUT_SIDE, dj : dj + OUT_SIDE, :] += cw[di, dj] * W1_img
    return W1f.reshape(IN_FLAT, HID)


def _pack_shard(x8, x16):
    """x8 [B_SHARD, NT8*112] fp8 (feature-major per tile), x16 likewise
    bf16 -> packed uint8 [KP, ROW_B_PER_COL*B_SHARD] per the chunk layout."""
    out = np.empty((KP, ROW_B_PER_COL * B_SHARD), np.uint8)
    v8 = np.ascontiguousarray(x8).view(np.uint8).reshape(B_SHARD, NT8, KP)
    v16 = (
        np.ascontiguousarray(x16).view(np.uint8).reshape(B_SHARD, NT16, KP, 2)
    )
    for c in range(len(CHUNKS)):
        j0, j1 = CHUNK_OFF[c], CHUNK_OFF[c + 1]
        cw = j1 - j0
        b0 = ROW_B_PER_COL * j0
        # fp8 tiles: [cw, NT8, KP] -> [KP, NT8, cw]
        blk8 = np.ascontiguousarray(v8[j0:j1].transpose(2, 1, 0)).reshape(
            KP, NT8 * cw
        )
        out[:, b0 : b0 + NT8 * cw] = blk8
        # bf16 tiles: [cw, NT16, KP, 2] -> [KP, NT16, cw, 2]
        blk16 = np.ascontiguousarray(v16[j0:j1].transpose(2, 1, 0, 3)).reshape(
            KP, 2 * NT16 * cw
        )
        out[:, b0 + NT8 * cw : b0 + ROW_B_PER_COL * cw] = blk16
    return out


def kernel(x, conv_w, W1, b1, W2, b2, W3, b3):
    global LAST_EXEC_NS, LAST_RESULTS
    x = np.asarray(x)
    W1f = _fold_conv_into_w1(conv_w, W1)

    # feature permutation: lowest-sensitivity features travel as fp8
    s2 = (W1f**2).sum(axis=1)
    order = np.argsort(s2, kind="stable")
    perm = np.concatenate([order[: NT8 * KP], np.sort(order[NT8 * KP :])])

    bf = ml_dtypes.bfloat16
    f8 = ml_dtypes.float8_e4m3
    w1_np = W1f[perm].astype(bf)
    w2_np = np.asarray(W2, np.float32).astype(bf)
    w3_np = np.asarray(W3, np.float32).astype(bf)

    wt_np = np.zeros((HID, WT_B), np.uint8)
    # w1 rows p<112: tile k at [k*256, (k+1)*256)
    w1_kpm = np.ascontiguousarray(
        w1_np.reshape(KT, KP, HID).transpose(1, 0, 2)
    )  # [112, 7, 128]
    wt_np[:KP, :W1_B] = w1_kpm.view(np.uint8).reshape(KP, W1_B)
    wt_np[:, W2_OFF : W2_OFF + HID * 2] = (
        np.ascontiguousarray(w2_np).view(np.uint8).reshape(HID, HID * 2)
    )
    wt_np[:, W3_OFF : W3_OFF + OUT * 2] = (
        np.ascontiguousarray(w3_np).view(np.uint8).reshape(HID, OUT * 2)
    )
    wt_np[:, B1_OFF : B1_OFF + 4] = (
        np.asarray(b1, np.float32).reshape(HID, 1).view(np.uint8)
    )
    wt_np[:, B2_OFF : B2_OFF + 4] = (
        np.asarray(b2, np.float32).reshape(HID, 1).view(np.uint8)
    )
    wt_np[:OUT, B3_OFF : B3_OFF + 4] = (
        np.asarray(b3, np.float32).reshape(OUT, 1).view(np.uint8)
    )

    if "prog" not in _compiled:
        _compiled["prog"] = _build_program()
    nc = _compiled["prog"]

    xp = x[:, perm]
    x8_all = xp[:, : NT8 * KP].astype(f8)
    x16_all = xp[:, NT8 * KP :].astype(bf)

    in_maps = []
    for c in range(N_CORES):
        sl = slice(c * B_SHARD, (c + 1) * B_SHARD)
        in_maps.append(
            {
                "xt": _pack_shard(x8_all[sl], x16_all[sl]),
                "wt": wt_np,
            }
        )

    trace = bool(int(os.environ.get("KERNEL_TRACE", "0")))
    res = run_bass_kernel_spmd(
        nc, in_maps, core_ids=list(range(N_CORES)), trace=trace
    )
    LAST_EXEC_NS = res.exec_time_ns
    LAST_RESULTS = res

    out = np.empty((B, OUT), np.float32)
    for c in range(N_CORES):
        out[c * B_SHARD : (c + 1) * B_SHARD, :] = res.results[c]["yt"].T
    return out


# revision 15
# speedup vs baseline: 1.1545x; 1.1545x over previous
"""Trainium2 Bass kernel for DigitConvolutionalModel.

Model: x[B,784] -> 3x3 valid conv (1 channel) -> flatten(676) -> FC(128)+relu
       -> FC(128)+relu (same W2 twice) -> FC(10).

Strategy:
  * Conv is linear, so it is folded into W1 on the host: W1f = C@W1 with C the
    [784,676] conv operator; the network becomes 4 dense layers.
  * Pure data parallel: batch 65536 split as 8192 per NeuronCore.
  * Activations stay transposed on chip ([hid partitions, batch free dim]);
    every layer is out = lhsT.T @ rhs with lhsT = weights.
  * The kernel is DMA-bound: 8 cores streaming simultaneously sustain only
    ~260 GB/s/core of HBM->SBUF, so input bytes are the roofline. Mixed
    precision cuts them: the PE accepts bf16 stationary x fp8 moving operands
    natively (verified exact), so the 448 input features with the smallest
    W1f row norms travel as fp8e4m3 and the sensitive 336 as bf16
    (0.714x bytes, rel err ~1.5e-2 vs 2e-2 budget). Weights stay bf16.
  * Per chunk, the 4 fp8 tiles + 3 bf16 tiles are packed into one uint8 DRAM
    row per partition (~5KB rows - measured near the per-row DMA sweet spot),
    one dma_start per chunk on the sync HWDGE ring; SBUF slices are
    bitcast back to fp8/bf16 for the matmuls.
  * Small first/last chunks shorten the serial dependency tail; outputs are
    flushed in 1024-col pieces on the sync ring (idle after input triggers).
  * 4-stage software pipeline as before: PE runs L1(s),L2(s-1),L3(s-2),
    L4(s-3); ACT does relu+bias for L1/L3, DVE for L2/L4.
"""

import os
import sys

sys.path.insert(0, "/opt/trn_rl_repo")

import ml_dtypes
import numpy as np

import concourse.bacc as bacc
import concourse.mybir as mybir
import concourse.tile as tile
from concourse.bass_utils import run_bass_kernel_spmd

B = 65536
IN_SIDE = 28
KSZ = 3
OUT_SIDE = IN_SIDE - KSZ + 1  # 26
FLAT = OUT_SIDE * OUT_SIDE  # 676
IN_FLAT = IN_SIDE * IN_SIDE  # 784
HID = 128
OUT = 10

N_CORES = 8
B_SHARD = B // N_CORES  # 8192
KP = 112  # feature-tile partition size (784 = 7*112)
KT = IN_FLAT // KP  # 7
NT8 = int(os.environ.get("KERNEL_NT8", "4"))  # tiles sent as fp8e4m3
NT16 = KT - NT8  # tiles sent as bf16
ROW_B_PER_COL = NT8 + 2 * NT16  # uint8 row bytes per batch column

# chunk schedule (batch columns per chunk): 1024-col bulk chunks give
# 10240B DMA rows (measured at the ~260GB/s per-row sweet spot) and hold
# exactly two 512-col matmul groups; small tail chunks shorten the final
# serial dependency chain
_chunks = os.environ.get("KERNEL_CHUNKS", "512,7x1024,256,256")
CHUNKS = []
for part in _chunks.split(","):
    if "x" in part:
        n, v = part.split("x")
        CHUNKS += [int(v)] * int(n)
    else:
        CHUNKS.append(int(part))
assert sum(CHUNKS) == B_SHARD
CHUNK_OFF = np.concatenate([[0], np.cumsum(CHUNKS)]).tolist()
# matmul groups: 512-col slices, never crossing a chunk boundary
GROUPS = []  # (chunk, local col offset, width)
for c, cw in enumerate(CHUNKS):
    off = 0
    while off < cw:
        w = min(512, cw - off)
        GROUPS.append((c, off, w))
        off += w

X_BUFS = int(os.environ.get("KERNEL_X_BUFS", "0"))  # 0 => all chunks resident

# packed weight row layout (bytes per partition row of the wt tensor):
# w1 [112p, 7 tiles x 256B] | w2 [128p, 256B] | w3 [128p, 20B] |
# b1 f32 | b2 f32 | b3 f32 (10p)
W1_B = KT * HID * 2  # 3584
W2_OFF = W1_B
W3_OFF = W2_OFF + HID * 2  # 3840
B1_OFF = W3_OFF + OUT * 2 + 16  # 3876 (16B pad keeps 4B alignment roomy)
B2_OFF = B1_OFF + 4
B3_OFF = B2_OFF + 4
WT_B = B3_OFF + 4  # 3888
H_BUFS = int(os.environ.get("KERNEL_H_BUFS", "4"))
PS_BUFS = int(os.environ.get("KERNEL_PS_BUFS", "8"))
OUT_FLUSH = int(os.environ.get("KERNEL_OUT_FLUSH", "1024"))
_skew = os.environ.get("KERNEL_SKEW", "2,4,6")
D2, D3, D4 = [int(v) for v in _skew.split(",")]

BF16 = mybir.dt.bfloat16
F8 = mybir.dt.float8e4
U8 = mybir.dt.uint8
F32 = mybir.dt.float32

LAST_EXEC_NS = None
LAST_RESULTS = None

_compiled = {}


def _build_program():
    n_chunks = len(CHUNKS)
    xt_bytes = ROW_B_PER_COL * B_SHARD

    nc = bacc.Bacc(
        "TRN2", target_bir_lowering=False, debug=False, num_devices=N_CORES
    )
    # packed input: chunk c spans byte cols
    # [ROW_B_PER_COL*off_c, ROW_B_PER_COL*off_{c+1}) ; within a chunk of C
    # cols the row is [4 fp8 tiles of C bytes | 3 bf16 tiles of 2C bytes]
    xt = nc.dram_tensor("xt", [KP, xt_bytes], U8, kind="ExternalInput")
    wt = nc.dram_tensor("wt", [HID, WT_B], U8, kind="ExternalInput")
    yt = nc.dram_tensor("yt", [OUT, B_SHARD], F32, kind="ExternalOutput")

    Relu = mybir.ActivationFunctionType.Relu
    add = mybir.AluOpType.add
    amax = mybir.AluOpType.max

    x_bufs = X_BUFS if X_BUFS > 0 else n_chunks
    with tile.TileContext(nc) as tc:
        with (
            tc.tile_pool(name="wpool", bufs=1) as wpool,
            tc.tile_pool(name="xpool", bufs=x_bufs) as xpool,
            tc.tile_pool(name="hpool", bufs=H_BUFS) as hpool,
            tc.tile_pool(name="opool", bufs=1) as opool,
            tc.tile_pool(name="ps1p", bufs=3, space="PSUM") as pp1,
            tc.tile_pool(name="ps2p", bufs=2, space="PSUM") as pp2,
            tc.tile_pool(name="ps3p", bufs=2, space="PSUM") as pp3,
            tc.tile_pool(name="ps4p", bufs=1, space="PSUM") as pp4,
        ):
            # all weights+biases packed in one [128, 3888B] tensor on the
            # scalar (ACT) HWDGE ring so it lands in parallel with chunk 0
            # streaming on the sync ring; the SWDGE path used before took
            # ~12us of tiny descriptors and gated the PE start
            wt_sb = wpool.tile([HID, WT_B], U8)
            nc.scalar.dma_start(out=wt_sb[:], in_=wt.ap())

            def w1_tile(k):
                return wt_sb[0:KP, k * HID * 2 : (k + 1) * HID * 2].bitcast(
                    BF16
                )

            w2_sb = wt_sb[:, W2_OFF : W2_OFF + HID * 2].bitcast(BF16)
            w3_sb = wt_sb[:, W3_OFF : W3_OFF + OUT * 2].bitcast(BF16)
            b1_sb = wt_sb[:, B1_OFF : B1_OFF + 4].bitcast(F32)
            b2_sb = wt_sb[:, B2_OFF : B2_OFF + 4].bitcast(F32)
            b3_sb = wt_sb[0:OUT, B3_OFF : B3_OFF + 4].bitcast(F32)

            yt_sb = opool.tile([OUT, B_SHARD], F32)

            xt_tiles = []
            for c in range(n_chunks):
                cw = CHUNKS[c]
                xt_sb = xpool.tile([KP, ROW_B_PER_COL * cw], U8, tag="xt")
                xt_tiles.append(xt_sb)
            for c in range(n_chunks):
                b0 = ROW_B_PER_COL * CHUNK_OFF[c]
                b1_ = ROW_B_PER_COL * CHUNK_OFF[c + 1]
                nc.sync.dma_start(out=xt_tiles[c][:], in_=xt.ap()[:, b0:b1_])

            def rhs_l1(g, k):
                """L1 moving operand: tile k of group g, bitcast view."""
                c, off, w = GROUPS[g]
                cw = CHUNKS[c]
                t = xt_tiles[c]
                if k < NT8:
                    lo = k * cw + off
                    return t[:, lo : lo + w].bitcast(F8)
                lo = NT8 * cw + 2 * ((k - NT8) * cw + off)
                return t[:, lo : lo + 2 * w].bitcast(BF16)

            def gw(g):
                return GROUPS[g][2]

            def goff(g):
                c, off, _ = GROUPS[g]
                return CHUNK_OFF[c] + off

            n_sub = len(GROUPS)
            h1t = {}
            h2t = {}
            h3t = {}
            ps = {}
            out_flushed = 0

            # 4-stage software pipeline: at skew step s the PE runs
            # L1(s), L2(s-D2), L3(s-D3), L4(s-D4) back-to-back; ACT/DVE
            # trail one stage behind each matmul.
            for s in range(n_sub + D4):
                if s < n_sub:
                    ps1 = pp1.tile([HID, gw(s)], F32, tag="ps1")
                    for k in range(KT):
                        nc.tensor.matmul(
                            ps1[:],
                            w1_tile(k),
                            rhs_l1(s, k),
                            start=(k == 0),
                            stop=(k == KT - 1),
                        )
                    ps[("1", s)] = ps1
                if s >= D2 and (s - D2) < n_sub:
                    j = s - D2
                    ps2 = pp2.tile([HID, gw(j)], F32, tag="ps2")
                    nc.tensor.matmul(
                        ps2[:], w2_sb, h1t[j][:], start=True, stop=True
                    )
                    ps[("2", j)] = ps2
                if s >= D3 and (s - D3) < n_sub:
                    j = s - D3
                    ps3 = pp3.tile([HID, gw(j)], F32, tag="ps3")
                    nc.tensor.matmul(
                        ps3[:], w2_sb, h2t[j][:], start=True, stop=True
                    )
                    ps[("3", j)] = ps3
                if s >= D4 and (s - D4) < n_sub:
                    j = s - D4
                    ps4 = pp4.tile([OUT, gw(j)], F32, tag="ps4")
                    nc.tensor.matmul(
                        ps4[:], w3_sb, h3t[j][:], start=True, stop=True
                    )
                    ps[("4", j)] = ps4

                # trailing activation/bias stages (other engines)
                if s < n_sub:
                    h1 = hpool.tile([HID, gw(s)], BF16, tag="h1")
                    nc.scalar.activation(
                        h1[:], ps[("1", s)][:], Relu, bias=b1_sb
                    )
                    h1t[s] = h1
                if s >= D2 and (s - D2) < n_sub:
                    j = s - D2
                    h2 = hpool.tile([HID, gw(j)], BF16, tag="h2")
                    nc.vector.tensor_scalar(
                        out=h2[:],
                        in0=ps[("2", j)][:],
                        scalar1=b2_sb,
                        scalar2=0.0,
                        op0=add,
                        op1=amax,
                    )
                    h2t[j] = h2
                if s >= D3 and (s - D3) < n_sub:
                    j = s - D3
                    h3 = hpool.tile([HID, gw(j)], BF16, tag="h3")
                    nc.scalar.activation(
                        h3[:], ps[("3", j)][:], Relu, bias=b2_sb
                    )
                    h3t[j] = h3
                if s >= D4 and (s - D4) < n_sub:
                    j = s - D4
                    j0 = goff(j)
                    nc.vector.tensor_scalar(
                        out=yt_sb[:, j0 : j0 + gw(j)],
                        in0=ps[("4", j)][:],
                        scalar1=b3_sb,
                        scalar2=None,
                        op0=add,
                    )
                    # flush finished cols on the sync ring (idle once the
                    # input triggers are all issued)
                    done = j0 + gw(j)
                    if done - out_flushed >= OUT_FLUSH or j == n_sub - 1:
                        nc.sync.dma_start(
                            out=yt.ap()[:, out_flushed:done],
                            in_=yt_sb[:, out_flushed:done],
                        )
                        out_flushed = done

    nc.compile()
    return nc


def _fold_conv_into_w1(conv_w, W1):
    """W1f[784,128] such that x @ W1f == conv(x).flatten @ W1."""
    W1_img = np.asarray(W1, np.float64).reshape(OUT_SIDE, OUT_SIDE, HID)
    cw = np.asarray(conv_w, np.float64).reshape(KSZ, KSZ)
    W1f = np.zeros((IN_SIDE, IN_SIDE, HID), np.float64)
    for di in range(KSZ):
        for dj in range(KSZ):
            W1f[di : di + OUT_SIDE, dj : dj + OUT_SIDE, :] += cw[di, dj] * W1_img
    return W1f.reshape(IN_FLAT, HID)


def _pack_shard(x8, x16):
    """x8 [B_SHARD, NT8*112] fp8 (feature-major per tile), x16 likewise
    bf16 -> packed uint8 [KP, ROW_B_PER_COL*B_SHARD] per the chunk layout."""
    out = np.empty((KP, ROW_B_PER_COL * B_SHARD), np.uint8)
    v8 = np.ascontiguousarray(x8).view(np.uint8).reshape(B_SHARD, NT8, KP)
    v16 = (
        np.ascontiguousarray(x16).view(np.uint8).reshape(B_SHARD, NT16, KP, 2)
    )
    for c in range(len(CHUNKS)):
        j0, j1 = CHUNK_OFF[c], CHUNK_OFF[c + 1]
        cw = j1 - j0
        b0 = ROW_B_PER_COL * j0
        # fp8 tiles: [cw, NT8, KP] -> [KP, NT8, cw]
        blk8 = np.ascontiguousarray(v8[j0:j1].transpose(2, 1, 0)).reshape(
            KP, NT8 * cw
        )
        out[:, b0 : b0 + NT8 * cw] = blk8
        # bf16 tiles: [cw, NT16, KP, 2] -> [KP, NT16, cw, 2]
        blk16 = np.ascontiguousarray(v16[j0:j1].transpose(2, 1, 0, 3)).reshape(
            KP, 2 * NT16 * cw
        )
        out[:, b0 + NT8 * cw : b0 + ROW_B_PER_COL * cw] = blk16
    return out


def kernel(x, conv_w, W1, b1, W2, b2, W3, b3):
    global LAST_EXEC_NS, LAST_RESULTS
    x = np.asarray(x)
    W1f = _fold_conv_into_w1(conv_w, W1)

    # feature permutation: lowest-sensitivity features travel as fp8
    s2 = (W1f**2).sum(axis=1)
    order = np.argsort(s2, kind="stable")
    perm = np.concatenate([order[: NT8 * KP], np.sort(order[NT8 * KP :])])

    bf = ml_dtypes.bfloat16
    f8 = ml_dtypes.float8_e4m3
    w1_np = W1f[perm].astype(bf)
    w2_np = np.asarray(W2, np.float32).astype(bf)
    w3_np = np.asarray(W3, np.float32).astype(bf)

    wt_np = np.zeros((HID, WT_B), np.uint8)
    # w1 rows p<112: tile k at [k*256, (k+1)*256)
    w1_kpm = np.ascontiguousarray(
        w1_np.reshape(KT, KP, HID).transpose(1, 0, 2)
    )  # [112, 7, 128]
    wt_np[:KP, :W1_B] = w1_kpm.view(np.uint8).reshape(KP, W1_B)
    wt_np[:, W2_OFF : W2_OFF + HID * 2] = (
        np.ascontiguousarray(w2_np).view(np.uint8).reshape(HID, HID * 2)
    )
    wt_np[:, W3_OFF : W3_OFF + OUT * 2] = (
        np.ascontiguousarray(w3_np).view(np.uint8).reshape(HID, OUT * 2)
    )
    wt_np[:, B1_OFF : B1_OFF + 4] = (
        np.asarray(b1, np.float32).reshape(HID, 1).view(np.uint8)
    )
    wt_np[:, B2_OFF : B2_OFF + 4] = (
        np.asarray(b2, np.float32).reshape(HID, 1).view(np.uint8)
    )
    wt_np[:OUT, B3_OFF : B3_OFF + 4] = (
        np.asarray(b3, np.float32).reshape(OUT, 1).view(np.uint8)
    )

    if "prog" not in _compiled:
        _compiled["prog"] = _build_program()
    nc = _compiled["prog"]

    xp = x[:, perm]
    x8_all = xp[:, : NT8 * KP].astype(f8)
    x16_all = xp[:, NT8 * KP :].astype(bf)

    in_maps = []
    for c in range(N_CORES):
        sl = slice(c * B_SHARD, (c + 1) * B_SHARD)
        in_maps.append(
            {
                "xt": _pack_shard(x8_all[sl], x16_all[sl]),
                "wt": wt_np,
            }
        )

    trace = bool(int(os.environ.get("KERNEL_TRACE", "0")))
    res = run_bass_kernel_spmd(
        nc, in_maps, core_ids=list(range(N_CORES)), trace=trace
    )
    LAST_EXEC_NS = res.exec_time_ns
    LAST_RESULTS = res

    out = np.empty((B, OUT), np.float32)
    for c in range(N_CORES):
        out[c * B_SHARD : (c + 1) * B_SHARD, :] = res.results[c]["yt"].T
    return out


# revision 17
# speedup vs baseline: 1.1596x; 1.0044x over previous
"""Trainium2 Bass kernel for DigitConvolutionalModel.

Model: x[B,784] -> 3x3 valid conv (1 channel) -> flatten(676) -> FC(128)+relu
       -> FC(128)+relu (same W2 twice) -> FC(10).

Strategy:
  * Conv is linear, so it is folded into W1 on the host: W1f = C@W1 with C the
    [784,676] conv operator; the network becomes 4 dense layers.
  * Pure data parallel: batch 65536 split as 8192 per NeuronCore.
  * Activations stay transposed on chip ([hid partitions, batch free dim]);
    every layer is out = lhsT.T @ rhs with lhsT = weights.
  * The kernel is DMA-bound: 8 cores streaming simultaneously sustain only
    ~260 GB/s/core of HBM->SBUF, so input bytes are the roofline. Mixed
    precision cuts them: the PE accepts bf16 stationary x fp8 moving operands
    natively (verified exact), so the 448 input features with the smallest
    W1f row norms travel as fp8e4m3 and the sensitive 336 as bf16
    (0.714x bytes, rel err ~1.5e-2 vs 2e-2 budget). Weights stay bf16.
  * Per chunk, the 4 fp8 tiles + 3 bf16 tiles are packed into one uint8 DRAM
    row per partition (~5KB rows - measured near the per-row DMA sweet spot),
    one dma_start per chunk on the sync HWDGE ring; SBUF slices are
    bitcast back to fp8/bf16 for the matmuls.
  * Small first/last chunks shorten the serial dependency tail; outputs are
    flushed in 1024-col pieces on the sync ring (idle after input triggers).
  * 4-stage software pipeline as before: PE runs L1(s),L2(s-1),L3(s-2),
    L4(s-3); ACT does relu+bias for L1/L3, DVE for L2/L4.
"""

import os
import sys

sys.path.insert(0, "/opt/trn_rl_repo")

import ml_dtypes
import numpy as np

import concourse.bacc as bacc
import concourse.mybir as mybir
import concourse.tile as tile
from concourse.bass_utils import run_bass_kernel_spmd

B = 65536
IN_SIDE = 28
KSZ = 3
OUT_SIDE = IN_SIDE - KSZ + 1  # 26
FLAT = OUT_SIDE * OUT_SIDE  # 676
IN_FLAT = IN_SIDE * IN_SIDE  # 784
HID = 128
OUT = 10

N_CORES = 8
B_SHARD = B // N_CORES  # 8192
KP = 112  # feature-tile partition size (784 = 7*112)
KT = IN_FLAT // KP  # 7
NT8 = int(os.environ.get("KERNEL_NT8", "4"))  # tiles sent as fp8e4m3
NT16 = KT - NT8  # tiles sent as bf16
ROW_B_PER_COL = NT8 + 2 * NT16  # uint8 row bytes per batch column

# chunk schedule (batch columns per chunk): 1024-col bulk chunks give
# 10240B DMA rows (measured at the ~260GB/s per-row sweet spot) and hold
# exactly two 512-col matmul groups; small tail chunks shorten the final
# serial dependency chain
_chunks = os.environ.get("KERNEL_CHUNKS", "512,7x1024,256,256")
CHUNKS = []
for part in _chunks.split(","):
    if "x" in part:
        n, v = part.split("x")
        CHUNKS += [int(v)] * int(n)
    else:
        CHUNKS.append(int(part))
assert sum(CHUNKS) == B_SHARD
CHUNK_OFF = np.concatenate([[0], np.cumsum(CHUNKS)]).tolist()
FLUSH_ENG = os.environ.get("KERNEL_FLUSH_ENG", "sync")
GW = int(os.environ.get("KERNEL_GW", "512"))
_pb = os.environ.get("KERNEL_PSB", "3,2,2,1")
PSB = [int(v) for v in _pb.split(",")]
# matmul groups: GW-col slices, never crossing a chunk boundary
GROUPS = []  # (chunk, local col offset, width)
for c, cw in enumerate(CHUNKS):
    off = 0
    while off < cw:
        w = min(GW, cw - off)
        GROUPS.append((c, off, w))
        off += w

X_BUFS = int(os.environ.get("KERNEL_X_BUFS", "0"))  # 0 => all chunks resident

# packed weight row layout (bytes per partition row of the wt tensor):
# w1 [112p, 7 tiles x 256B] | w2 [128p, 256B] | w3 [128p, 20B] |
# b1 f32 | b2 f32 | b3 f32 (10p)
W1_B = KT * HID * 2  # 3584
W2_OFF = W1_B
W3_OFF = W2_OFF + HID * 2  # 3840
B1_OFF = W3_OFF + OUT * 2 + 16  # 3876 (16B pad keeps 4B alignment roomy)
B2_OFF = B1_OFF + 4
B3_OFF = B2_OFF + 4
WT_B = B3_OFF + 4  # 3888
H_BUFS = int(os.environ.get("KERNEL_H_BUFS", "4"))
PS_BUFS = int(os.environ.get("KERNEL_PS_BUFS", "8"))
OUT_FLUSH = int(os.environ.get("KERNEL_OUT_FLUSH", "1024"))
_skew = os.environ.get("KERNEL_SKEW", "2,4,6")
D2, D3, D4 = [int(v) for v in _skew.split(",")]

BF16 = mybir.dt.bfloat16
F8 = mybir.dt.float8e4
U8 = mybir.dt.uint8
F32 = mybir.dt.float32

LAST_EXEC_NS = None
LAST_RESULTS = None

_compiled = {}


def _build_program():
    n_chunks = len(CHUNKS)
    xt_bytes = ROW_B_PER_COL * B_SHARD

    nc = bacc.Bacc(
        "TRN2", target_bir_lowering=False, debug=False, num_devices=N_CORES
    )
    # packed input: chunk c spans byte cols
    # [ROW_B_PER_COL*off_c, ROW_B_PER_COL*off_{c+1}) ; within a chunk of C
    # cols the row is [4 fp8 tiles of C bytes | 3 bf16 tiles of 2C bytes]
    xt = nc.dram_tensor("xt", [KP, xt_bytes], U8, kind="ExternalInput")
    wt = nc.dram_tensor("wt", [HID, WT_B], U8, kind="ExternalInput")
    yt = nc.dram_tensor("yt", [OUT, B_SHARD], F32, kind="ExternalOutput")

    Relu = mybir.ActivationFunctionType.Relu
    add = mybir.AluOpType.add
    amax = mybir.AluOpType.max

    x_bufs = X_BUFS if X_BUFS > 0 else n_chunks
    with tile.TileContext(nc) as tc:
        with (
            tc.tile_pool(name="wpool", bufs=1) as wpool,
            tc.tile_pool(name="xpool", bufs=x_bufs) as xpool,
            tc.tile_pool(name="hpool", bufs=H_BUFS) as hpool,
            tc.tile_pool(name="opool", bufs=1) as opool,
            tc.tile_pool(name="ps1p", bufs=PSB[0], space="PSUM") as pp1,
            tc.tile_pool(name="ps2p", bufs=PSB[1], space="PSUM") as pp2,
            tc.tile_pool(name="ps3p", bufs=PSB[2], space="PSUM") as pp3,
            tc.tile_pool(name="ps4p", bufs=PSB[3], space="PSUM") as pp4,
        ):
            # all weights+biases packed in one [128, 3888B] tensor on the
            # scalar (ACT) HWDGE ring so it lands in parallel with chunk 0
            # streaming on the sync ring; the SWDGE path used before took
            # ~12us of tiny descriptors and gated the PE start
            wt_sb = wpool.tile([HID, WT_B], U8)
            nc.scalar.dma_start(out=wt_sb[:], in_=wt.ap())

            def w1_tile(k):
                return wt_sb[0:KP, k * HID * 2 : (k + 1) * HID * 2].bitcast(
                    BF16
                )

            w2_sb = wt_sb[:, W2_OFF : W2_OFF + HID * 2].bitcast(BF16)
            w3_sb = wt_sb[:, W3_OFF : W3_OFF + OUT * 2].bitcast(BF16)
            b1_sb = wt_sb[:, B1_OFF : B1_OFF + 4].bitcast(F32)
            b2_sb = wt_sb[:, B2_OFF : B2_OFF + 4].bitcast(F32)
            b3_sb = wt_sb[0:OUT, B3_OFF : B3_OFF + 4].bitcast(F32)

            yt_sb = opool.tile([OUT, B_SHARD], F32)

            xt_tiles = []
            for c in range(n_chunks):
                cw = CHUNKS[c]
                xt_sb = xpool.tile([KP, ROW_B_PER_COL * cw], U8, tag="xt")
                xt_tiles.append(xt_sb)
            for c in range(n_chunks):
                b0 = ROW_B_PER_COL * CHUNK_OFF[c]
                b1_ = ROW_B_PER_COL * CHUNK_OFF[c + 1]
                nc.sync.dma_start(out=xt_tiles[c][:], in_=xt.ap()[:, b0:b1_])

            def rhs_l1(g, k):
                """L1 moving operand: tile k of group g, bitcast view."""
                c, off, w = GROUPS[g]
                cw = CHUNKS[c]
                t = xt_tiles[c]
                if k < NT8:
                    lo = k * cw + off
                    return t[:, lo : lo + w].bitcast(F8)
                lo = NT8 * cw + 2 * ((k - NT8) * cw + off)
                return t[:, lo : lo + 2 * w].bitcast(BF16)

            def gw(g):
                return GROUPS[g][2]

            def goff(g):
                c, off, _ = GROUPS[g]
                return CHUNK_OFF[c] + off

            n_sub = len(GROUPS)
            h1t = {}
            h2t = {}
            h3t = {}
            ps = {}
            out_flushed = 0

            # 4-stage software pipeline: at skew step s the PE runs
            # L1(s), L2(s-D2), L3(s-D3), L4(s-D4) back-to-back; ACT/DVE
            # trail one stage behind each matmul.
            for s in range(n_sub + D4):
                if s < n_sub:
                    ps1 = pp1.tile([HID, gw(s)], F32, tag="ps1")
                    for k in range(KT):
                        nc.tensor.matmul(
                            ps1[:],
                            w1_tile(k),
                            rhs_l1(s, k),
                            start=(k == 0),
                            stop=(k == KT - 1),
                        )
                    ps[("1", s)] = ps1
                if s >= D2 and (s - D2) < n_sub:
                    j = s - D2
                    ps2 = pp2.tile([HID, gw(j)], F32, tag="ps2")
                    nc.tensor.matmul(
                        ps2[:], w2_sb, h1t[j][:], start=True, stop=True
                    )
                    ps[("2", j)] = ps2
                if s >= D3 and (s - D3) < n_sub:
                    j = s - D3
                    ps3 = pp3.tile([HID, gw(j)], F32, tag="ps3")
                    nc.tensor.matmul(
                        ps3[:], w2_sb, h2t[j][:], start=True, stop=True
                    )
                    ps[("3", j)] = ps3
                if s >= D4 and (s - D4) < n_sub:
                    j = s - D4
                    ps4 = pp4.tile([OUT, gw(j)], F32, tag="ps4")
                    nc.tensor.matmul(
                        ps4[:], w3_sb, h3t[j][:], start=True, stop=True
                    )
                    ps[("4", j)] = ps4

                # trailing activation/bias stages (other engines)
                if s < n_sub:
                    h1 = hpool.tile([HID, gw(s)], BF16, tag="h1")
                    nc.scalar.activation(
                        h1[:], ps[("1", s)][:], Relu, bias=b1_sb
                    )
                    h1t[s] = h1
                if s >= D2 and (s - D2) < n_sub:
                    j = s - D2
                    h2 = hpool.tile([HID, gw(j)], BF16, tag="h2")
                    nc.vector.tensor_scalar(
                        out=h2[:],
                        in0=ps[("2", j)][:],
                        scalar1=b2_sb,
                        scalar2=0.0,
                        op0=add,
                        op1=amax,
                    )
                    h2t[j] = h2
                if s >= D3 and (s - D3) < n_sub:
                    j = s - D3
                    h3 = hpool.tile([HID, gw(j)], BF16, tag="h3")
                    nc.scalar.activation(
                        h3[:], ps[("3", j)][:], Relu, bias=b2_sb
                    )
                    h3t[j] = h3
                if s >= D4 and (s - D4) < n_sub:
                    j = s - D4
                    j0 = goff(j)
                    nc.vector.tensor_scalar(
                        out=yt_sb[:, j0 : j0 + gw(j)],
                        in0=ps[("4", j)][:],
                        scalar1=b3_sb,
                        scalar2=None,
                        op0=add,
                    )
                    # flush finished cols on the sync ring (idle once the
                    # input triggers are all issued)
                    done = j0 + gw(j)
                    if done - out_flushed >= OUT_FLUSH or j == n_sub - 1:
                        feng = getattr(nc, FLUSH_ENG)
                        feng.dma_start(
                            out=yt.ap()[:, out_flushed:done],
                            in_=yt_sb[:, out_flushed:done],
                        )
                        out_flushed = done

    nc.compile()
    return nc


def _fold_conv_into_w1(conv_w, W1):
    """W1f[784,128] such that x @ W1f == conv(x).flatten @ W1."""
    W1_img = np.asarray(W1, np.float64).reshape(OUT_SIDE, OUT_SIDE, HID)
    cw = np.asarray(conv_w, np.float64).reshape(KSZ, KSZ)
    W1f = np.zeros((IN_SIDE, IN_SIDE, HID), np.float64)
    for di in range(KSZ):
        for dj in range(KSZ):
            W1f[di : di + OUT_SIDE, dj : dj + OUT_SIDE, :] += cw[di, dj] * W1_img
    return W1f.reshape(IN_FLAT, HID)


def _pack_shard(x8, x16):
    """x8 [B_SHARD, NT8*112] fp8 (feature-major per tile), x16 likewise
    bf16 -> packed uint8 [KP, ROW_B_PER_COL*B_SHARD] per the chunk layout."""
    out = np.empty((KP, ROW_B_PER_COL * B_SHARD), np.uint8)
    v8 = np.ascontiguousarray(x8).view(np.uint8).reshape(B_SHARD, NT8, KP)
    v16 = (
        np.ascontiguousarray(x16).view(np.uint8).reshape(B_SHARD, NT16, KP, 2)
    )
    for c in range(len(CHUNKS)):
        j0, j1 = CHUNK_OFF[c], CHUNK_OFF[c + 1]
        cw = j1 - j0
        b0 = ROW_B_PER_COL * j0
        # fp8 tiles: [cw, NT8, KP] -> [KP, NT8, cw]
        blk8 = np.ascontiguousarray(v8[j0:j1].transpose(2, 1, 0)).reshape(
            KP, NT8 * cw
        )
        out[:, b0 : b0 + NT8 * cw] = blk8
        # bf16 tiles: [cw, NT16, KP, 2] -> [KP, NT16, cw, 2]
        blk16 = np.ascontiguousarray(v16[j0:j1].transpose(2, 1, 0, 3)).reshape(
            KP, 2 * NT16 * cw
        )
        out[:, b0 + NT8 * cw : b0 + ROW_B_PER_COL * cw] = blk16
    return out


def kernel(x, conv_w, W1, b1, W2, b2, W3, b3):
    global LAST_EXEC_NS, LAST_RESULTS
    x = np.asarray(x)
    W1f = _fold_conv_into_w1(conv_w, W1)

    # feature permutation: lowest-sensitivity features travel as fp8
    s2 = (W1f**2).sum(axis=1)
    order = np.argsort(s2, kind="stable")
    perm = np.concatenate([order[: NT8 * KP], np.sort(order[NT8 * KP :])])

    bf = ml_dtypes.bfloat16
    f8 = ml_dtypes.float8_e4m3
    w1_np = W1f[perm].astype(bf)
    w2_np = np.asarray(W2, np.float32).astype(bf)
    w3_np = np.asarray(W3, np.float32).astype(bf)

    wt_np = np.zeros((HID, WT_B), np.uint8)
    # w1 rows p<112: tile k at [k*256, (k+1)*256)
    w1_kpm = np.ascontiguousarray(
        w1_np.reshape(KT, KP, HID).transpose(1, 0, 2)
    )  # [112, 7, 128]
    wt_np[:KP, :W1_B] = w1_kpm.view(np.uint8).reshape(KP, W1_B)
    wt_np[:, W2_OFF : W2_OFF + HID * 2] = (
        np.ascontiguousarray(w2_np).view(np.uint8).reshape(HID, HID * 2)
    )
    wt_np[:, W3_OFF : W3_OFF + OUT * 2] = (
        np.ascontiguousarray(w3_np).view(np.uint8).reshape(HID, OUT * 2)
    )
    wt_np[:, B1_OFF : B1_OFF + 4] = (
        np.asarray(b1, np.float32).reshape(HID, 1).view(np.uint8)
    )
    wt_np[:, B2_OFF : B2_OFF + 4] = (
        np.asarray(b2, np.float32).reshape(HID, 1).view(np.uint8)
    )
    wt_np[:OUT, B3_OFF : B3_OFF + 4] = (
        np.asarray(b3, np.float32).reshape(OUT, 1).view(np.uint8)
    )

    if "prog" not in _compiled:
        _compiled["prog"] = _build_program()
    nc = _compiled["prog"]

    xp = x[:, perm]
    x8_all = xp[:, : NT8 * KP].astype(f8)
    x16_all = xp[:, NT8 * KP :].astype(bf)

    in_maps = []
    for c in range(N_CORES):
        sl = slice(c * B_SHARD, (c + 1) * B_SHARD)
        in_maps.append(
            {
                "xt": _pack_shard(x8_all[sl], x16_all[sl]),
                "wt": wt_np,
            }
        )

    trace = bool(int(os.environ.get("KERNEL_TRACE", "0")))
    res = run_bass_kernel_spmd(
        nc, in_maps, core_ids=list(range(N_CORES)), trace=trace
    )
    LAST_EXEC_NS = res.exec_time_ns
    LAST_RESULTS = res

    out = np.empty((B, OUT), np.float32)
    for c in range(N_CORES):
        out[c * B_SHARD : (c + 1) * B_SHARD, :] = res.results[c]["yt"].T
    return out
